# revision 10
# baseline (speedup 1.0000x reference)
"""Trainium2 Bass kernel for nn_HashEncoder (instant-NGP style hash-grid encoder).

Contract: kernel(inputs, embeddings) -> [1M, 32] f32.
Sharding: data-parallel over points, 8 cores; full 57MB table in each core's HBM.
Device work per core: normalize coords, per level compute corner hash indices on
DVE (uint32 math, 5-bit decomposed multiplies — products stay exact through the
DVE float multiply path), then gather corner rows with BATCHED indirect DMAs
(GOFF offsets per partition per instruction instead of 1 — the SWDGE fixed cost
of ~1us/instruction dominated the old per-(corner,column) form), trilinear-
interpolate via broadcast-AP multiplies + segmented tensor_reduce. Per-level
table base is folded into the DMA via element_offset.
"""
import sys

if "/opt/trn_rl_repo" not in sys.path:
    sys.path.insert(0, "/opt/trn_rl_repo")

import numpy as np

# ---- problem constants (hardcoded per harness contract) ----
D, L, C, H = 3, 16, 2, 16
T = 2 ** 19
BOUND = 1.0
PRIMES = (1, 2654435761, 805459861)
B_FULL = 1_000_000
N_CORES = 8


def _make_offsets():
    offs, o = [0], 0
    for l in range(L):
        res = H * (2 ** l)
        o += min(T, (res + 1) ** D)
        offs.append(o)
    return offs


OFFSETS = _make_offsets()
N_PARAMS = OFFSETS[-1]  # 7131219

# per-core point layout: NPC points = 128 partitions x NC cols, point(p, j) = p*NC + j
NC_COLS = 977
NPC = 128 * NC_COLS          # 125056
B_PAD = NPC * N_CORES        # 1000448
CN_TILE = 192                # cols per SBUF tile
MASK19 = 0x7FFFF
N_QUEUES = 4                 # SWDGE queues to spread gathers across


def _build(npc, nc_cols, cn_tile, levels):
    import concourse.bass as bass
    import concourse.tile as tile
    from concourse import bacc, mybir

    dt = mybir.dt
    Alu = mybir.AluOpType
    P = 128

    nc = bacc.Bacc("TRN2", target_bir_lowering=False, debug=False,
                   enable_asserts=False, num_devices=N_CORES,
                   num_swdge_queues=N_QUEUES)
    pts_d = nc.dram_tensor("pts", [npc, 3], dt.float32, kind="ExternalInput")
    emb_d = nc.dram_tensor("emb", [N_PARAMS, C], dt.float32, kind="ExternalInput")
    nout = 2 * len(levels)
    out_d = nc.dram_tensor("out", [npc, nout], dt.float32, kind="ExternalOutput")

    pts_v = pts_d.ap().rearrange("(p n) d -> p n d", p=P)   # [128, nc_cols, 3]
    out_v = out_d.ap().rearrange("(p n) c -> p n c", p=P)   # [128, nc_cols, nout]

    col_tiles = []
    jb = 0
    while jb < nc_cols:
        cn = min(cn_tile, nc_cols - jb)
        col_tiles.append((jb, cn))
        jb += cn

    with tile.TileContext(nc) as tc:
        with tc.tile_pool(name="sb", bufs=2) as sb, \
             tc.tile_pool(name="consts", bufs=1) as cpool:

            # uint32 constant tiles [P, 1] (int immediates aren't supported)
            _consts = {}

            def cu(val):
                if val not in _consts:
                    t = cpool.tile([P, 1], dt.uint32, tag=f"c{val}")
                    nc.vector.memset(t[:, :], val)
                    _consts[val] = t
                return _consts[val][:, :1]

            def ibc(val, shape_free):
                # broadcast [P,1] uint32 const along free dims
                return cu(val).to_broadcast([P] + shape_free)

            # [P, 2] constant pairs (one value per hash dim d in {1,2}),
            # broadcast along the column axis
            _cpairs = {}

            def cpair(v1, v2, shape_free_cn):
                key = (v1, v2)
                if key not in _cpairs:
                    t = cpool.tile([P, 2], dt.uint32, tag=f"cp{v1}_{v2}")
                    nc.vector.memset(t[:, 0:1], v1)
                    nc.vector.memset(t[:, 1:2], v2)
                    _cpairs[key] = t
                t = _cpairs[key]
                f = t[:, :]
                return bass.AP(f.tensor, f.offset,
                               [f.ap[0], [1, 2], [0, shape_free_cn]])

            _qrr = [0]

            def gather(idx_tile, n_off, feats_tile, elems_per_idx, level):
                """Gather: idx_tile [P, n_off] int32 row indices (level-local),
                feats_tile flat [P, n_off*elems_per_idx] f32. The SWDGE ucode
                only supports one offset per partition per instruction, so
                issue n_off [128,1]-offset indirect DMAs, round-robin across
                the SWDGE queues."""
                it = idx_tile.bitcast(dt.int32)
                ft = feats_tile
                for g in range(n_off):
                    off_ap = bass.AP(it.tensor, it.offset + g,
                                     [it.ap[0], [1, 1]])
                    out_ap = bass.AP(ft.tensor, ft.offset + g * elems_per_idx,
                                     [ft.ap[0], [1, elems_per_idx]])
                    inst = nc.gpsimd.indirect_dma_start(
                        out=out_ap,
                        out_offset=None,
                        in_=emb_d[:, :],
                        in_offset=bass.IndirectOffsetOnAxis(ap=off_ap, axis=0),
                        element_offset=2 * OFFSETS[level],
                    )
                    if N_QUEUES > 1:
                        q = _qrr[0] = (_qrr[0] + 1) % N_QUEUES
                        inst.ins.queue = f"qPoolDynamic{q or ''}"

            for ti, (jb, cn) in enumerate(col_tiles):
                pts_t = sb.tile([P, cn, 3], dt.float32, tag="pts")
                nc.sync.dma_start(out=pts_t[:, :, :], in_=pts_v[:, jb:jb + cn, :])

                # xn = clip((pts+1)*0.5, 0, 1)
                xn = sb.tile([P, cn, 3], dt.float32, tag="xn")
                nc.vector.tensor_scalar(
                    out=xn[:, :, :], in0=pts_t[:, :, :], scalar1=0.5, scalar2=0.5,
                    op0=Alu.mult, op1=Alu.add)
                nc.vector.tensor_scalar(
                    out=xn[:, :, :], in0=xn[:, :, :], scalar1=1.0, scalar2=0.0,
                    op0=Alu.min, op1=Alu.max)

                outt = sb.tile([P, cn, nout], dt.float32, tag="outt")

                for li, l in enumerate(levels):
                    res = H * (2 ** l)
                    size = OFFSETS[l + 1] - OFFSETS[l]
                    dense = (res + 1) ** D <= size

                    pos3 = sb.tile([P, cn, 3], dt.float32, tag="pos3")
                    nc.vector.tensor_scalar(
                        out=pos3[:, :, :], in0=xn[:, :, :], scalar1=float(res),
                        scalar2=None, op0=Alu.mult)

                    # floor: r = rint(pos); rf = f32(r); gt = (rf > pos); pgf = rf-gt
                    pgu = sb.tile([P, cn, 3], dt.uint32, tag="pgu")
                    rf = sb.tile([P, cn, 3], dt.float32, tag="rf")
                    gt = sb.tile([P, cn, 3], dt.float32, tag="gtf")
                    nc.vector.tensor_copy(out=pgu[:, :, :], in_=pos3[:, :, :])
                    nc.vector.tensor_copy(out=rf[:, :, :], in_=pgu[:, :, :])
                    nc.vector.tensor_tensor(
                        out=gt[:, :, :], in0=rf[:, :, :], in1=pos3[:, :, :],
                        op=Alu.is_gt)
                    nc.vector.tensor_tensor(
                        out=rf[:, :, :], in0=rf[:, :, :], in1=gt[:, :, :],
                        op=Alu.subtract)
                    nc.vector.tensor_scalar(
                        out=rf[:, :, :], in0=rf[:, :, :], scalar1=float(res - 1),
                        scalar2=None, op0=Alu.min)
                    # f2[0]=1-frac, f2[1]=frac ; frac = pos - pgf
                    f2 = sb.tile([P, 2, 3, cn], dt.float32, tag="f2")
                    frac = sb.tile([P, cn, 3], dt.float32, tag="frac")
                    nc.vector.tensor_tensor(
                        out=frac[:, :, :], in0=pos3[:, :, :], in1=rf[:, :, :],
                        op=Alu.subtract)
                    nc.vector.tensor_copy(out=pgu[:, :, :], in_=rf[:, :, :])
                    # f2[1, d, :] = frac[:, :, d] (free-dim transpose copy)
                    f2f = f2[:, :, :, :]
                    frf = frac[:, :, :]
                    nc.vector.tensor_copy(
                        out=bass.AP(f2f.tensor, f2f.offset + 3 * cn,
                                    [f2f.ap[0], [cn, 3], [1, cn]]),
                        in_=bass.AP(frf.tensor, frf.offset,
                                    [frf.ap[0], [1, 3], [3, cn]]))
                    # f2[0, d, :] = 1 - frac = (frac * -1) - (-1)
                    nc.vector.tensor_scalar(
                        out=bass.AP(f2f.tensor, f2f.offset,
                                    [f2f.ap[0], [cn, 3], [1, cn]]),
                        in0=bass.AP(frf.tensor, frf.offset,
                                    [frf.ap[0], [1, 3], [3, cn]]),
                        scalar1=-1.0, scalar2=-1.0,
                        op0=Alu.mult, op1=Alu.subtract)

                    pg_f = pgu[:, :, :]
                    # view of (y, z) columns as [P, d=2, cn]
                    pd_ap = bass.AP(pg_f.tensor, pg_f.offset + 1,
                                    [pg_f.ap[0], [1, 2], [3, cn]])

                    if dense:
                        s1, s2 = res + 1, (res + 1) ** 2
                        # t_yz[c, d, :] : c=corner bit, d in {y,z}
                        t_yz = sb.tile([P, 2, 2, cn], dt.uint32, tag="tyz")
                        nc.vector.tensor_tensor(
                            out=t_yz[:, 0, :, :], in0=pd_ap,
                            in1=cpair(s1, s2, cn), op=Alu.mult)
                        nc.vector.tensor_tensor(
                            out=t_yz[:, 1, :, :], in0=t_yz[:, 0, :, :],
                            in1=cpair(s1, s2, cn), op=Alu.add)
                        # idx4[by, bz] = x + ty[by] + tz[bz]   [P, 4, cn]
                        ty_f = t_yz[:, :, :, :]
                        xy_d = sb.tile([P, 2, cn], dt.uint32, tag="xyd")
                        x_ap = bass.AP(pg_f.tensor, pg_f.offset,
                                       [pg_f.ap[0], [0, 2], [3, cn]])
                        ty_ap = bass.AP(ty_f.tensor, ty_f.offset,
                                        [ty_f.ap[0], [2 * cn, 2], [1, cn]])
                        nc.vector.tensor_tensor(
                            out=xy_d[:, :, :], in0=x_ap, in1=ty_ap, op=Alu.add)
                        idx4 = sb.tile([P, 4, cn], dt.uint32, tag="idx4")
                        xyd_f = xy_d[:, :, :]
                        idx4_f = idx4[:, :, :]
                        tz_ap = bass.AP(ty_f.tensor, ty_f.offset + cn,
                                        [ty_f.ap[0], [0, 2], [2 * cn, 2], [1, cn]])
                        in_xyd = bass.AP(xyd_f.tensor, xyd_f.offset,
                                         [xyd_f.ap[0], [cn, 2], [0, 2], [1, cn]])
                        o_idx4 = bass.AP(idx4_f.tensor, idx4_f.offset,
                                         [idx4_f.ap[0], [2 * cn, 2], [cn, 2],
                                          [1, cn]])
                        nc.vector.tensor_tensor(
                            out=o_idx4, in0=in_xyd, in1=tz_ap, op=Alu.add)

                        # gather pairs: rows (idx, idx+1), 4 f32 per offset
                        feats_dn = sb.tile([P, 4, cn, 4], dt.float32,
                                           tag="featsd")
                        gather(idx4[:, :, :], 4 * cn,
                               feats_dn[:, :, :, :], 4, l)
                    else:
                        # hash: acc_d = (pgu_d * PRIMES[d]) mod 2^19 via 5-bit
                        # chunks of y/z; products < 2^24 stay exact through the
                        # DVE float multiply path. Both dims in one [P,2,cn] op.
                        nbits = min(l + 5, 20)
                        nch = -(-nbits // 5)
                        t_yz = sb.tile([P, 2, 2, cn], dt.uint32, tag="tyz")
                        acc = t_yz[:, 0, :, :]
                        nib = sb.tile([P, 2, cn], dt.uint32, tag="nib")
                        for jc in range(nch):
                            p1k = (PRIMES[1] << (5 * jc)) & MASK19
                            p2k = (PRIMES[2] << (5 * jc)) & MASK19
                            if jc == 0:
                                nc.vector.tensor_tensor(
                                    out=nib[:, :, :], in0=pd_ap,
                                    in1=ibc(31, [2, cn]), op=Alu.bitwise_and)
                            else:
                                nc.vector.tensor_tensor(
                                    out=nib[:, :, :], in0=pd_ap,
                                    in1=ibc(5 * jc, [2, cn]),
                                    op=Alu.logical_shift_right)
                                nc.vector.tensor_tensor(
                                    out=nib[:, :, :], in0=nib[:, :, :],
                                    in1=ibc(31, [2, cn]), op=Alu.bitwise_and)
                            nc.vector.tensor_tensor(
                                out=nib[:, :, :], in0=nib[:, :, :],
                                in1=cpair(p1k, p2k, cn), op=Alu.mult)
                            if jc == 0:
                                nc.vector.tensor_tensor(
                                    out=acc, in0=nib[:, :, :],
                                    in1=ibc(MASK19, [2, cn]),
                                    op=Alu.bitwise_and)
                            else:
                                nc.vector.tensor_tensor(
                                    out=nib[:, :, :], in0=nib[:, :, :],
                                    in1=ibc(MASK19, [2, cn]),
                                    op=Alu.bitwise_and)
                                nc.vector.tensor_tensor(
                                    out=acc, in0=acc, in1=nib[:, :, :],
                                    op=Alu.add)
                        # corner+1 terms: acc + (prime mod 2^19)
                        nc.vector.tensor_tensor(
                            out=t_yz[:, 1, :, :], in0=acc,
                            in1=cpair(PRIMES[1] & MASK19, PRIMES[2] & MASK19,
                                      cn),
                            op=Alu.add)

                        # x terms [P, 2, cn]: x, x+1
                        tx = sb.tile([P, 2, cn], dt.uint32, tag="tx")
                        x_ap = bass.AP(pg_f.tensor, pg_f.offset,
                                       [pg_f.ap[0], [1, 1], [3, cn]])
                        nc.vector.tensor_copy(
                            out=tx[:, 0:1, :], in_=x_ap)
                        nc.vector.tensor_tensor(
                            out=tx[:, 1:2, :], in0=x_ap,
                            in1=cu(1).to_broadcast([P, 1, cn]), op=Alu.add)

                        # combine: k = bx*4 + by*2 + bz
                        ty_f = t_yz[:, :, :, :]
                        tx_f = tx[:, :, :]
                        xy = sb.tile([P, 2, 2, cn], dt.uint32, tag="xy")
                        in_x = bass.AP(tx_f.tensor, tx_f.offset,
                                       [tx_f.ap[0], [cn, 2], [0, 2], [1, cn]])
                        in_y = bass.AP(ty_f.tensor, ty_f.offset,
                                       [ty_f.ap[0], [0, 2], [2 * cn, 2], [1, cn]])
                        nc.vector.tensor_tensor(
                            out=xy[:, :, :, :], in0=in_x, in1=in_y,
                            op=Alu.bitwise_xor)
                        idx8 = sb.tile([P, 8, cn], dt.uint32, tag="idx8")
                        xy_f = xy[:, :, :, :]
                        idx8_f = idx8[:, :, :]
                        for bz in range(2):
                            in_xy = bass.AP(xy_f.tensor, xy_f.offset,
                                            [xy_f.ap[0], [2 * cn, 2], [cn, 2],
                                             [1, cn]])
                            in_z = bass.AP(ty_f.tensor,
                                           ty_f.offset + bz * 2 * cn + cn,
                                           [ty_f.ap[0], [0, 2], [0, 2], [1, cn]])
                            o_z = bass.AP(idx8_f.tensor, idx8_f.offset + bz * cn,
                                          [idx8_f.ap[0], [4 * cn, 2], [2 * cn, 2],
                                           [1, cn]])
                            nc.vector.tensor_tensor(
                                out=o_z, in0=in_xy, in1=in_z, op=Alu.bitwise_xor)
                        nc.vector.tensor_tensor(
                            out=idx8[:, :, :], in0=idx8[:, :, :],
                            in1=ibc(MASK19, [8, cn]), op=Alu.bitwise_and)

                        feats = sb.tile([P, 8, cn, 2], dt.float32, tag="feats")
                        gather(idx8[:, :, :], 8 * cn,
                               feats[:, :, :, :], 2, l)

                    # ---- weights: w8[k] = fx_bx * fy_by * fz_bz ----
                    f2_f = f2[:, :, :, :]
                    xyw = sb.tile([P, 2, 2, cn], dt.float32, tag="xyw")
                    wx = bass.AP(f2_f.tensor, f2f.offset,
                                 [f2_f.ap[0], [3 * cn, 2], [0, 2], [1, cn]])
                    wy = bass.AP(f2_f.tensor, f2f.offset + cn,
                                 [f2_f.ap[0], [0, 2], [3 * cn, 2], [1, cn]])
                    nc.vector.tensor_tensor(
                        out=xyw[:, :, :, :], in0=wx, in1=wy, op=Alu.mult)
                    w8 = sb.tile([P, 8, cn], dt.float32, tag="w8")
                    xyw_f = xyw[:, :, :, :]
                    w8_f = w8[:, :, :]
                    for bz in range(2):
                        in_xyw = bass.AP(xyw_f.tensor, xyw_f.offset,
                                         [xyw_f.ap[0], [2 * cn, 2], [cn, 2],
                                          [1, cn]])
                        wz = bass.AP(f2_f.tensor,
                                     f2f.offset + bz * 3 * cn + 2 * cn,
                                     [f2_f.ap[0], [0, 2], [0, 2], [1, cn]])
                        o_w = bass.AP(w8_f.tensor, w8_f.offset + bz * cn,
                                      [w8_f.ap[0], [4 * cn, 2], [2 * cn, 2],
                                       [1, cn]])
                        nc.vector.tensor_tensor(out=o_w, in0=in_xyw, in1=wz,
                                                op=Alu.mult)

                    # ---- interp: per channel, prod = w8*feats_c ; reduce over k
                    outt_f = outt[:, :, :]
                    for c in range(2):
                        prod = sb.tile([P, cn, 8], dt.float32, tag="prod")
                        if dense:
                            fd_f = feats_dn[:, :, :, :]
                            for bx in range(2):
                                w_v = bass.AP(w8_f.tensor,
                                              w8_f.offset + bx * 4 * cn,
                                              [w8_f.ap[0], [1, cn], [cn, 4]])
                                f_v = bass.AP(fd_f.tensor,
                                              fd_f.offset + bx * 2 + c,
                                              [fd_f.ap[0], [4, cn], [4 * cn, 4]])
                                o_v = bass.AP(prod[:, :, :].tensor,
                                              prod[:, :, :].offset + bx * 4,
                                              [prod[:, :, :].ap[0], [8, cn],
                                               [1, 4]])
                                nc.vector.tensor_tensor(
                                    out=o_v, in0=w_v, in1=f_v, op=Alu.mult)
                        else:
                            feats_f = feats[:, :, :, :]
                            w_v = bass.AP(w8_f.tensor, w8_f.offset,
                                          [w8_f.ap[0], [1, cn], [cn, 8]])
                            f_v = bass.AP(feats_f.tensor, feats_f.offset + c,
                                          [feats_f.ap[0], [2, cn], [2 * cn, 8]])
                            nc.vector.tensor_tensor(
                                out=prod[:, :, :], in0=w_v, in1=f_v, op=Alu.mult)
                        res_v = bass.AP(outt_f.tensor,
                                        outt_f.offset + li * 2 + c,
                                        [outt_f.ap[0], [nout, cn]])
                        nc.vector.tensor_reduce(
                            out=res_v, in_=prod[:, :, :],
                            axis=mybir.AxisListType.X, op=Alu.add)

                nc.sync.dma_start(out=out_v[:, jb:jb + cn, :], in_=outt[:, :, :])

    nc.compile()
    return nc


_BUILD_CACHE = {}


def _get_nc(npc, nc_cols, cn_tile, levels):
    key = (npc, nc_cols, cn_tile, tuple(levels))
    if key not in _BUILD_CACHE:
        _BUILD_CACHE[key] = _build(npc, nc_cols, cn_tile, levels)
    return _BUILD_CACHE[key]


def kernel(inputs: np.ndarray, embeddings: np.ndarray, _trace=False) -> np.ndarray:
    from concourse.bass_utils import run_bass_kernel_spmd

    inputs = np.ascontiguousarray(inputs, dtype=np.float32)
    embeddings = np.ascontiguousarray(embeddings, dtype=np.float32)
    B = inputs.shape[0]

    pts_pad = np.zeros((B_PAD, 3), dtype=np.float32)
    pts_pad[:B] = inputs
    nc = _get_nc(NPC, NC_COLS, CN_TILE, list(range(L)))
    in_maps = [dict(pts=pts_pad[c * NPC:(c + 1) * NPC], emb=embeddings)
               for c in range(N_CORES)]
    import time as _time
    _t0 = _time.time()
    r = run_bass_kernel_spmd(nc, in_maps, core_ids=list(range(N_CORES)),
                             trace=False)
    kernel._last_wall_s = _time.time() - _t0
    out = np.concatenate([r.results[c]["out"] for c in range(N_CORES)], axis=0)
    kernel._last_exec_ns = r.exec_time_ns
    return out[:B]
